# revision 53
# baseline (speedup 1.0000x reference)
"""Fused multi-head attention on 8 TRN2 NeuronCores.

Problem: x[2,2048,1024] -> q,k,v = x@W.T+b (16 heads x 64), softmax(q k^T/8) v,
then out @ Wp.T + bp.

Sharding: data-parallel over batch (2) x tensor-parallel over heads (4 ranks x
4 heads = 256 dims, Megatron-style).  Core c handles batch c//4, head-rank c%4.
The proj partial sums are reduced on the host (numpy), and the v-bias and
proj-bias are folded into one host-side vector bp_eff = bv @ Wp.T + bp.

Per-core layouts (host pre-transposes/pre-tiles):
  xT  [1024, 2048]  x[b].T
  wqT/wkT/wvT [128, 8*256]  W.T slice pre-tiled so partition p holds all 8
                            contraction tiles contiguously
  wpT [256, 1024]           Wp.T rows for this rank's 256 dims
  bq/bk [256, 1]
  outT [1024, 2048] partial (x[b] @ ..).T, missing bv/bp contributions

Kernel design (v2): the kernel is ACT(exp)-bound in steady state -- 16.8M exps
per core at 1 elem/lane/cycle @1.2GHz is ~142us of Scalar-engine work in
[128,1024] tiles.  Everything else is arranged so the Scalar engine runs that
stream densely from ~6us:
  - groups of 2 key-blocks; emission order per group is
    [scores row2-packed] -> [exp x2 on ACT] -> [trailing PV + denominator
    matmuls col2-packed for the previous group] -> [full-mode filler
    projections]; 3 PE tiling-mode switches per group instead of per block.
  - x is DMA'd column-first (all 8 k-tiles' cols 0:512 first) so the q/k
    projections for the first chunk finish early; a tiny activation at t=0
    pre-loads the exp table set, and a few dummy matmuls warm the PE HAM.
  - denominators: adjacent exp tiles pair-summed on the DVE (never gpsimd --
    it is 3x slower and contends for the same SBUF port), lagged one group,
    then ones-matmuls accumulate into a per-pair pd PSUM tile.
  - PSUM: score tiles tag "s" ring 2x[128,1024] (4 banks) shared with the v
    projection, filler projections tag "fp" ring 1x[128,1024] (2 banks,
    filler windows serialized), po+pd tag "po" ring 2x[128,512] (2 banks).
  - attnT = po * reciprocal_approx_fast(pd); outT += wpT.T @ attnT.
"""

import numpy as np

DIM = 1024
N_TOK = 2048
N_HEADS_LOC = 4       # heads per core
D_LOC = 256           # local q/k/v dims per core
SCALE = 64 ** -0.5
P = 128
CH = 512              # n-chunk (moving free dim)
NCH = N_TOK // CH     # 4
KT = DIM // P         # 8 contraction tiles for qkv/proj
MB = N_TOK // P       # 16 key blocks
N_CORES = 8
NG = 8                # groups of 2 key blocks per (chunk, head-pair)

_NC_CACHE = {}


def build_nc(dt_mm_name="bfloat16"):
    import math

    import concourse.mybir as mybir
    import concourse.tile as tile
    from concourse import bacc
    from concourse.bass import ts

    f32 = mybir.dt.float32
    dt_mm = getattr(mybir.dt, dt_mm_name)
    Exp = mybir.ActivationFunctionType.Exp

    nc = bacc.Bacc("TRN2", target_bir_lowering=False, debug=False,
                   num_devices=N_CORES)
    # xq: chunk-major x layout -- xq[c][p, kt*CH + t] = x.T[128*kt + p, CH*c + t]
    # so each of the 4 chunk tiles is one 1MB DMA with 8KB-contiguous rows.
    # DMA-efficiency-driven layouts: big transfers with >=8KB contiguous per
    # partition (4KB-row transfers measured only ~125-180 GB/s on the SWDGE
    # queue).  boot = [wk | wq | x chunk 0], wvx1 = [wv | x chunk 1].
    boot = nc.dram_tensor("boot", [P, 2 * KT * D_LOC + KT * CH], dt_mm,
                          kind="ExternalInput").ap()
    wvx1 = nc.dram_tensor("wvx1", [P, KT * D_LOC + KT * CH], dt_mm,
                          kind="ExternalInput").ap()
    xq2 = nc.dram_tensor("xq2", [P, KT * CH], dt_mm, kind="ExternalInput").ap()
    xq3 = nc.dram_tensor("xq3", [P, KT * CH], dt_mm, kind="ExternalInput").ap()
    wpT = nc.dram_tensor("wpT", [D_LOC, DIM], dt_mm, kind="ExternalInput").ap()
    bq = nc.dram_tensor("bq", [D_LOC, 1], f32, kind="ExternalInput").ap()
    bk = nc.dram_tensor("bk", [D_LOC, 1], f32, kind="ExternalInput").ap()
    outT = nc.dram_tensor("outT", [DIM, N_TOK], dt_mm, kind="ExternalOutput").ap()

    with tile.TileContext(nc) as tc:
        with (
            tc.tile_pool(name="const", bufs=1) as const,
            tc.tile_pool(name="work", bufs=2) as work,
            tc.tile_pool(name="psum", bufs=2, space="PSUM") as psum,
            tc.tile_pool(name="psum_o", bufs=2, space="PSUM") as psum_o,
        ):
            # ---- persistent SBUF tiles ----
            WSZ = KT * D_LOC
            boot_sb = const.tile([P, 2 * WSZ + KT * CH], dt_mm, tag="boot",
                                 name="boot_sb")
            wvx1_sb = const.tile([P, WSZ + KT * CH], dt_mm, tag="wvx1",
                                 name="wvx1_sb")
            w_kq = boot_sb[:, 0:2 * WSZ]
            w_tiles = {"k": boot_sb[:, 0:WSZ],
                       "q": boot_sb[:, WSZ:2 * WSZ],
                       "v": wvx1_sb[:, 0:WSZ]}
            xq_sb = [
                boot_sb[:, 2 * WSZ:],
                wvx1_sb[:, WSZ:],
                const.tile([P, KT * CH], dt_mm, tag="xq2", name="xq2_sb"),
                const.tile([P, KT * CH], dt_mm, tag="xq3", name="xq3_sb"),
            ]
            bias_sb = {}
            for name in ("q", "k"):
                bias_sb[name] = [const.tile([P, 1], f32, tag=f"b{name}{mt}",
                                            name=f"b{name}{mt}")
                                 for mt in range(D_LOC // P)]
            wp_sb = [const.tile([P, DIM], dt_mm, tag=f"wp{i}", name=f"wp{i}")
                     for i in range(D_LOC // P)]
            ones_sb = const.tile([P, 64], dt_mm, tag="ones")
            warm_sb = const.tile([P, CH], dt_mm, tag="warm")
            qk_sb = {}
            for name in ("q", "k"):
                qk_sb[name] = [
                    const.tile([P, N_TOK], dt_mm, tag=f"{name}T{mt}",
                               name=f"{name}T{mt}")
                    for mt in range(D_LOC // P)
                ]
            vpk_sb = [
                const.tile([P, N_HEADS_LOC, 64], dt_mm, tag=f"vp{nt}",
                           name=f"vp{nt}")
                for nt in range(MB)
            ]
            w_sb = {name: [w_tiles[name][:, ts(i, D_LOC)] for i in range(KT)]
                    for name in ("k", "q", "v")}

            def x_chunk(kt, chunk):
                """x.T[128*kt:128*kt+128, CH*chunk:CH*(chunk+1)] view."""
                return xq_sb[chunk][:, ts(kt, CH)]

            def x_block(kt, nt):
                """x.T[128*kt:.., 128*nt:128*nt+128] (one key block)."""
                c, r = divmod(nt, NCH)
                return xq_sb[c][:, kt * CH + r * P:kt * CH + (r + 1) * P]

            # ---- t=0: exp-table primer + PE HAM warm-up ----
            nc.vector.memset(ones_sb[:], 1.0)
            nc.vector.memset(warm_sb[:], 0.0)
            prim = work.tile([P, 8], f32, tag="prim")
            nc.scalar.activation(prim[:], ones_sb[:, 0:8], Exp)
            wps = psum.tile([P, 1024], f32, tag="s", name="warmps")
            for i in range(20):         # bridge from framework start to the
                nc.tensor.matmul(       # boot DMA so the HAM never throttles
                    wps[:, 0:CH], lhsT=warm_sb[:, 0:P],
                    rhs=warm_sb[:], start=True, stop=True)

            # ---- DMA emission ----
            # Everything on the critical path goes on ONE queue in priority
            # order (two active queues round-robin at the SDMA level, which
            # starves whichever one holds the critical transfers), and every
            # big transfer is ~1MB with 8KB contiguous per partition.
            nc.gpsimd.dma_start(out=boot_sb[:], in_=boot[:])
            nc.gpsimd.dma_start(out=wvx1_sb[:], in_=wvx1[:])
            nc.gpsimd.dma_start(out=xq_sb[2][:], in_=xq2[:])
            nc.gpsimd.dma_start(out=xq_sb[3][:], in_=xq3[:])
            for i in range(D_LOC // P):
                nc.gpsimd.dma_start(out=wp_sb[i][:], in_=wpT[ts(i, P), :])
            # biases (tiny, consumed via DVE epilogues) on the other queue
            for name, src_ap in (("q", bq), ("k", bk)):
                for mt in range(D_LOC // P):
                    nc.sync.dma_start(out=bias_sb[name][mt][:],
                                      in_=src_ap[ts(mt, P), :])

            # ---- emission units ----
            def gen_qk(name, mt, h2):
                """q/k projection, one 512-col chunk per "fp" ring slot so
                the DVE epilogue of one chunk overlaps the next chunk's
                matmuls instead of serializing on a shared psum tile."""
                for half in range(2):
                    ps = psum.tile([P, CH], f32, tag="fp", bufs=2,
                                   name=f"fp_{name}{mt}{h2}{half}")
                    for kt in range(KT):
                        nc.tensor.matmul(
                            ps[:],
                            lhsT=w_sb[name][kt][:, ts(mt, P)],
                            rhs=x_chunk(kt, 2 * h2 + half),
                            start=(kt == 0), stop=(kt == KT - 1),
                        )
                        yield
                    nc.vector.tensor_scalar_add(
                        qk_sb[name][mt][:, ts(2 * h2 + half, CH)],
                        ps[:], bias_sb[name][mt][:],
                    )
                    yield

            def gen_v(nts):
                """v-projection groups; one key block per yield.  Shares the
                scores "s" psum ring (runs only while pair 0 is in flight)."""
                for nt in nts:
                    ps = psum.tile([P, 1024], f32, tag="s", name=f"ps_v{nt}")
                    for kt in range(KT):
                        nc.tensor.matmul(
                            ps[:, 0:D_LOC],
                            lhsT=x_block(kt, nt),
                            rhs=w_sb["v"][kt][:],
                            start=(kt == 0), stop=(kt == KT - 1),
                        )
                    nc.vector.tensor_copy(
                        vpk_sb[nt][:].rearrange("p h n -> p (h n)"),
                        ps[:, 0:D_LOC])
                    yield

            def gen_outproj(ch, tag="fp"):
                """Output projection for chunk ch; one 128-row mo slice per
                "fp"/"s" ring slot, evacuated per slice."""
                at_tiles = at_sb[ch]
                for mo in range(DIM // P):
                    pp = psum.tile([P, CH], f32, tag=tag, bufs=2,
                                   name=f"pp{ch}{mo}")
                    for dt_i in range(2):
                        nc.tensor.matmul(
                            pp[:],
                            lhsT=wp_sb[dt_i][:, ts(mo, P)],
                            rhs=at_tiles[dt_i][:],
                            start=(dt_i == 0), stop=(dt_i == 1),
                        )
                    yield
                    os_sb = work.tile([P, CH], dt_mm, tag="os", bufs=6,
                                      name=f"os{ch}{mo}")
                    nc.vector.tensor_copy(os_sb[:], pp[:])
                    nc.sync.dma_start(out=outT[ts(mo, P), ts(ch, CH)],
                                      in_=os_sb[:])
                    yield

            o3p = {}

            def gen_op3_partial():
                """First-head-pair half of outproj chunk 3, staged to SBUF
                during pair 7 so only the second half trails the last exp."""
                at0 = at_sb[3][0]
                for mo in range(DIM // P):
                    pp = psum.tile([P, CH], f32, tag="fp", bufs=2,
                                   name=f"p3a{mo}")
                    nc.tensor.matmul(pp[:], lhsT=wp_sb[0][:, ts(mo, P)],
                                     rhs=at0[:], start=True, stop=True)
                    yield
                    o3p[mo] = work.tile([P, CH], f32, tag="o3p", bufs=8,
                                        name=f"o3p{mo}")
                    nc.vector.tensor_copy(o3p[mo][:], pp[:])
                    yield

            def run_op3_tail():
                at1 = at_sb[3][1]
                for mo in range(DIM // P):
                    pp = psum.tile([P, CH], f32, tag="s", bufs=2,
                                   name=f"p3b{mo}")
                    nc.tensor.matmul(pp[:], lhsT=wp_sb[1][:, ts(mo, P)],
                                     rhs=at1[:], start=True, stop=True)
                    os_sb = work.tile([P, CH], dt_mm, tag="os", bufs=6,
                                      name=f"os3{mo}")
                    nc.vector.tensor_add(os_sb[:], pp[:], o3p[mo][:])
                    nc.sync.dma_start(out=outT[ts(mo, P), ts(3, CH)],
                                      in_=os_sb[:])

            def run(gen):
                for _ in gen:
                    pass

            # ---- preamble: k/q chunk-0 projections, interleaved so the two
            # "fp" ring slots pipeline (k h0 -> slot A, q h0 -> slot B) and
            # the first scores can go right after the two epilogues.
            kh = gen_qk("k", 0, 0)
            qh = gen_qk("q", 0, 0)
            for _ in range(8):
                next(kh)                # k chunk-0 matmuls
            for _ in range(8):
                next(qh)                # q chunk-0 matmuls
            next(kh)                    # k chunk-0 bias epilogue
            next(qh)                    # q chunk-0 bias epilogue
            v_gen = gen_v(list(range(MB)))

            # ---- filler schedule: (start_group, end_group, generator) ----
            # successive "fp" tenancies must be released in emission order.
            at_sb = {}
            fillers = [
                [0, 0, kh, 9],          # k chunk 1 (needed by scores of G1)
                [0, 7, v_gen, 16],
                [0, 3, qh, 9],          # q chunk 1 (needed by pair 1)
                [2, 5, gen_qk("k", 0, 1), 18],
                [5, 8, gen_qk("k", 1, 0), 18],
                [9, 12, gen_qk("k", 1, 1), 18],
                [13, 16, gen_qk("q", 1, 0), 18],
                [17, 20, gen_qk("q", 0, 1), 18],
                [21, 24, gen_qk("q", 1, 1), 18],
                [26, 31, None, 16],     # outproj 0 (created lazily)
                [33, 39, None, 16],     # outproj 1
                [49, 55, None, 16],     # outproj 2
                [57, 62, None, 16],     # outproj 3, first head-pair half
            ]
            op_for_window = {9: 0, 10: 1, 11: 2}

            def pump(G):
                for idx, f in enumerate(fillers):
                    s, e, gen, rem = f
                    if G < s or rem <= 0:
                        continue
                    if gen is None:
                        gen = f[2] = (gen_op3_partial() if idx == 12 else
                                      gen_outproj(op_for_window[idx]))
                    n = rem if G >= e else math.ceil(rem / (e - G + 1))
                    for _ in range(n):
                        if next(gen, StopIteration) is StopIteration:
                            f[3] = 0
                            break
                        f[3] -= 1

            # ---- main loop: 64 groups of 2 key blocks ----
            SEQ = [(0, 0), (1, 0), (0, 1), (1, 1),
                   (2, 0), (2, 1), (3, 0), (3, 1)]
            pts = {}
            pend = {}
            po_pd = {}
            for G in range(NCH * 2 * NG + 1):
                if G < NCH * 2 * NG:
                    p, g = divmod(G, NG)
                    c, hp = SEQ[p]
                    if g == 0:
                        po_pd[p] = (
                            psum_o.tile([P, CH], f32, tag="po", name=f"po{p}"),
                            psum_o.tile([P, CH], f32, tag="po", name=f"pd{p}"),
                        )
                    # leading: scores + exp for blocks 2g, 2g+1 (row2 packed)
                    for j in range(2):
                        mb = 2 * g + j
                        ps = psum.tile([P, 1024], f32, tag="s",
                                       name=f"s{p}_{g}{j}")
                        nc.tensor.matmul(
                            ps[:, 0:CH],
                            lhsT=qk_sb["k"][hp][0:64, ts(mb, P)],
                            rhs=qk_sb["q"][hp][0:64, ts(c, CH)],
                        )
                        nc.tensor.matmul(
                            ps[:, CH:1024],
                            lhsT=qk_sb["k"][hp][64:P, ts(mb, P)],
                            rhs=qk_sb["q"][hp][64:P, ts(c, CH)],
                        )
                        pt = work.tile([P, 1024], dt_mm, tag="pt", bufs=12,
                                       name=f"pt{p}_{g}{j}")
                        nc.scalar.activation(pt[:], ps[:], Exp, scale=SCALE)
                        pts[(G, j)] = pt
                    # denominator pair-sum on the DVE (consumed next group)
                    s2 = work.tile([P, 1024], dt_mm, tag="pts2", bufs=6,
                                   name=f"pts2_{p}_{g}")
                    nc.vector.tensor_add(s2[:], pts[(G, 0)][:], pts[(G, 1)][:])
                    pend[G] = s2
                if G >= 1:
                    # trailing: PV + pd for group G-1 (col2 packed)
                    G2 = G - 1
                    p2, g2 = divmod(G2, NG)
                    c2, hp2 = SEQ[p2]
                    po, pd = po_pd[p2]
                    for j in range(2):
                        mb = 2 * g2 + j
                        pt = pts.pop((G2, j))
                        st = (mb == 0)
                        sp = (mb == MB - 1)
                        nc.tensor.matmul(
                            po[0:64, :], lhsT=vpk_sb[mb][:, 2 * hp2, :],
                            rhs=pt[:, 0:CH], start=st, stop=sp,
                        )
                        nc.tensor.matmul(
                            po[64:P, :], lhsT=vpk_sb[mb][:, 2 * hp2 + 1, :],
                            rhs=pt[:, CH:1024], start=st, stop=sp,
                        )
                    if g2 >= 1:
                        s2p = pend.pop(G2 - 1)
                        nc.tensor.matmul(
                            pd[0:64, :], lhsT=ones_sb[:],
                            rhs=s2p[:, 0:CH], start=(g2 == 1), stop=False,
                        )
                        nc.tensor.matmul(
                            pd[64:P, :], lhsT=ones_sb[:],
                            rhs=s2p[:, CH:1024], start=(g2 == 1), stop=False,
                        )
                    if g2 == NG - 1:
                        # pair p2 done: last pd group + normalize
                        s2p = pend.pop(G2)
                        nc.tensor.matmul(
                            pd[0:64, :], lhsT=ones_sb[:],
                            rhs=s2p[:, 0:CH], start=False, stop=True,
                        )
                        nc.tensor.matmul(
                            pd[64:P, :], lhsT=ones_sb[:],
                            rhs=s2p[:, CH:1024], start=False, stop=True,
                        )
                        del po_pd[p2]
                        rec = work.tile([P, CH], f32, tag="bc", bufs=4,
                                        name=f"rec{p2}")
                        nc.vector.reciprocal_approx_fast(rec[:], pd[:])
                        at = work.tile([P, CH], dt_mm, tag="at", bufs=4,
                                       name=f"at{p2}")
                        nc.vector.tensor_mul(at[:], po[:], rec[:])
                        at_sb.setdefault(c2, []).append(at)
                if G < NCH * 2 * NG:
                    # fillers last: scores stay at the front of the PE queue
                    # so the exp stream never starves (filler windows end one
                    # group before their outputs' first consumers)
                    pump(G)
            # tail: only the second head-pair half of outproj chunk 3 remains
            run_op3_tail()

    nc.compile()
    return nc


def _get_nc():
    if "nc" not in _NC_CACHE:
        _NC_CACHE["nc"] = build_nc(DT_MM_NAME)
    return _NC_CACHE["nc"]


def make_in_maps(x, Wq, bq, Wk, bk, Wv, bv, Wp, bp, dt_mm_name="bfloat16"):
    """Shard full inputs into 8 per-core input maps."""
    f = np.float32
    if dt_mm_name == "bfloat16":
        import ml_dtypes
        mmt = ml_dtypes.bfloat16
    else:
        mmt = np.float32
    x = np.asarray(x, f)
    # chunk-major x: xq[c][p, kt*CH + t] = x[b].T[128*kt + p, CH*c + t]
    xqs = []
    for b in range(x.shape[0]):
        xb = np.ascontiguousarray(x[b].T).astype(mmt)          # [1024, 2048]
        xqs.append(np.ascontiguousarray(
            xb.reshape(KT, P, NCH, CH).transpose(2, 1, 0, 3)
              .reshape(NCH * P, KT * CH)))
    WqT = np.asarray(Wq, f).T
    WkT = np.asarray(Wk, f).T
    WvT = np.asarray(Wv, f).T
    WpT = np.asarray(Wp, f).T
    CAT = np.concatenate
    def pretile(w):
        # [1024, 256] -> [128, 8*256]: partition p holds all 8 k-tiles
        # contiguously so DMA descriptors are 4KB DRAM runs
        return np.ascontiguousarray(
            w.reshape(KT, P, D_LOC).transpose(1, 0, 2).reshape(P, KT * D_LOC)
        ).astype(mmt)

    in_maps = []
    for c in range(N_CORES):
        b, r = divmod(c, 4)
        sl = slice(D_LOC * r, D_LOC * (r + 1))
        xqb = xqs[b]
        in_maps.append({
            "boot": np.ascontiguousarray(
                CAT([pretile(WkT[:, sl]), pretile(WqT[:, sl]),
                     xqb[0:P]], axis=1)),
            "wvx1": np.ascontiguousarray(
                CAT([pretile(WvT[:, sl]), xqb[P:2 * P]], axis=1)),
            "xq2": np.ascontiguousarray(xqb[2 * P:3 * P]),
            "xq3": np.ascontiguousarray(xqb[3 * P:4 * P]),
            "wpT": np.ascontiguousarray(WpT[sl, :]).astype(mmt),
            "bq": np.asarray(bq, f)[sl].reshape(D_LOC, 1).copy(),
            "bk": np.asarray(bk, f)[sl].reshape(D_LOC, 1).copy(),
        })
    return in_maps


def assemble_output(results, Wv, bv, Wp, bp):
    """Sum TP partials, transpose back, add folded biases."""
    f = np.float32
    bp_eff = np.asarray(bv, f) @ np.asarray(Wp, f).T + np.asarray(bp, f)
    out = np.empty((2, N_TOK, DIM), f)
    for b in range(2):
        acc = results[4 * b]["outT"].astype(f)
        for r in range(1, 4):
            acc = acc + results[4 * b + r]["outT"]
        out[b] = acc.T + bp_eff
    return out


DT_MM_NAME = "bfloat16"


def kernel(x, Wq, bq, Wk, bk, Wv, bv, Wp, bp):
    from concourse.bass_utils import run_bass_kernel_spmd
    nc = _get_nc()
    in_maps = make_in_maps(x, Wq, bq, Wk, bk, Wv, bv, Wp, bp, DT_MM_NAME)
    res = run_bass_kernel_spmd(nc, in_maps, list(range(N_CORES)))
    return assemble_output(res.results, Wv, bv, Wp, bp)


# revision 54
# speedup vs baseline: 1.1837x; 1.1837x over previous
"""Fused multi-head attention on 8 TRN2 NeuronCores.

Problem: x[2,2048,1024] -> q,k,v = x@W.T+b (16 heads x 64), softmax(q k^T/8) v,
then out @ Wp.T + bp.

Sharding: data-parallel over batch (2) x tensor-parallel over heads (4 ranks x
4 heads = 256 dims, Megatron-style).  Core c handles batch c//4, head-rank c%4.
The proj partial sums are reduced on the host (numpy), and the v-bias and
proj-bias are folded into one host-side vector bp_eff = bv @ Wp.T + bp.

Per-core layouts (host pre-transposes/pre-tiles):
  xT  [1024, 2048]  x[b].T
  wqT/wkT/wvT [128, 8*256]  W.T slice pre-tiled so partition p holds all 8
                            contraction tiles contiguously
  wpT [256, 1024]           Wp.T rows for this rank's 256 dims
  bq/bk [256, 1]
  outT [1024, 2048] partial (x[b] @ ..).T, missing bv/bp contributions

Kernel design (v2): the kernel is ACT(exp)-bound in steady state -- 16.8M exps
per core at 1 elem/lane/cycle @1.2GHz is ~142us of Scalar-engine work in
[128,1024] tiles.  Everything else is arranged so the Scalar engine runs that
stream densely from ~6us:
  - groups of 2 key-blocks; emission order per group is
    [scores row2-packed] -> [exp x2 on ACT] -> [trailing PV + denominator
    matmuls col2-packed for the previous group] -> [full-mode filler
    projections]; 3 PE tiling-mode switches per group instead of per block.
  - x is DMA'd column-first (all 8 k-tiles' cols 0:512 first) so the q/k
    projections for the first chunk finish early; a tiny activation at t=0
    pre-loads the exp table set, and a few dummy matmuls warm the PE HAM.
  - denominators: adjacent exp tiles pair-summed on the DVE (never gpsimd --
    it is 3x slower and contends for the same SBUF port), lagged one group,
    then ones-matmuls accumulate into a per-pair pd PSUM tile.
  - PSUM: score tiles tag "s" ring 2x[128,1024] (4 banks) shared with the v
    projection, filler projections tag "fp" ring 1x[128,1024] (2 banks,
    filler windows serialized), po+pd tag "po" ring 2x[128,512] (2 banks).
  - attnT = po * reciprocal_approx_fast(pd); outT += wpT.T @ attnT.
"""

import numpy as np

DIM = 1024
N_TOK = 2048
N_HEADS_LOC = 4       # heads per core
D_LOC = 256           # local q/k/v dims per core
SCALE = 64 ** -0.5
P = 128
CH = 512              # n-chunk (moving free dim)
NCH = N_TOK // CH     # 4
KT = DIM // P         # 8 contraction tiles for qkv/proj
MB = N_TOK // P       # 16 key blocks
N_CORES = 8
NG = 8                # groups of 2 key blocks per (chunk, head-pair)

_NC_CACHE = {}


def build_nc(dt_mm_name="bfloat16"):
    import math

    import concourse.mybir as mybir
    import concourse.tile as tile
    from concourse import bacc
    from concourse.bass import ts

    f32 = mybir.dt.float32
    dt_mm = getattr(mybir.dt, dt_mm_name)
    Exp = mybir.ActivationFunctionType.Exp

    nc = bacc.Bacc("TRN2", target_bir_lowering=False, debug=False,
                   num_devices=N_CORES)
    # xq: chunk-major x layout -- xq[c][p, kt*CH + t] = x.T[128*kt + p, CH*c + t]
    # so each of the 4 chunk tiles is one 1MB DMA with 8KB-contiguous rows.
    # DMA-efficiency-driven layouts: big transfers with >=8KB contiguous per
    # partition (4KB-row transfers measured only ~125-180 GB/s on the SWDGE
    # queue).  boot = [wk | wq | x chunk 0], wvx1 = [wv | x chunk 1].
    boot = nc.dram_tensor("boot", [P, 2 * KT * D_LOC + KT * CH], dt_mm,
                          kind="ExternalInput").ap()
    wvx1 = nc.dram_tensor("wvx1", [P, KT * D_LOC + KT * CH], dt_mm,
                          kind="ExternalInput").ap()
    xq2 = nc.dram_tensor("xq2", [P, KT * CH], dt_mm, kind="ExternalInput").ap()
    xq3 = nc.dram_tensor("xq3", [P, KT * CH], dt_mm, kind="ExternalInput").ap()
    wpT = nc.dram_tensor("wpT", [D_LOC, DIM], dt_mm, kind="ExternalInput").ap()
    bq = nc.dram_tensor("bq", [D_LOC, 1], f32, kind="ExternalInput").ap()
    bk = nc.dram_tensor("bk", [D_LOC, 1], f32, kind="ExternalInput").ap()
    outT = nc.dram_tensor("outT", [DIM, N_TOK], dt_mm, kind="ExternalOutput").ap()

    with tile.TileContext(nc) as tc:
        with (
            tc.tile_pool(name="const", bufs=1) as const,
            tc.tile_pool(name="work", bufs=2) as work,
            tc.tile_pool(name="psum", bufs=2, space="PSUM") as psum,
            tc.tile_pool(name="psum_o", bufs=2, space="PSUM") as psum_o,
        ):
            # ---- persistent SBUF tiles ----
            WSZ = KT * D_LOC
            boot_sb = const.tile([P, 2 * WSZ + KT * CH], dt_mm, tag="boot",
                                 name="boot_sb")
            wvx1_sb = const.tile([P, WSZ + KT * CH], dt_mm, tag="wvx1",
                                 name="wvx1_sb")
            w_kq = boot_sb[:, 0:2 * WSZ]
            w_tiles = {"k": boot_sb[:, 0:WSZ],
                       "q": boot_sb[:, WSZ:2 * WSZ],
                       "v": wvx1_sb[:, 0:WSZ]}
            xq_sb = [
                boot_sb[:, 2 * WSZ:],
                wvx1_sb[:, WSZ:],
                const.tile([P, KT * CH], dt_mm, tag="xq2", name="xq2_sb"),
                const.tile([P, KT * CH], dt_mm, tag="xq3", name="xq3_sb"),
            ]
            bias_sb = {}
            for name in ("q", "k"):
                bias_sb[name] = [const.tile([P, 1], f32, tag=f"b{name}{mt}",
                                            name=f"b{name}{mt}")
                                 for mt in range(D_LOC // P)]
            wp_sb = [const.tile([P, DIM], dt_mm, tag=f"wp{i}", name=f"wp{i}")
                     for i in range(D_LOC // P)]
            ones_sb = const.tile([P, 64], dt_mm, tag="ones")
            warm_sb = const.tile([P, CH], dt_mm, tag="warm")
            qk_sb = {}
            for name in ("q", "k"):
                qk_sb[name] = [
                    const.tile([P, N_TOK], dt_mm, tag=f"{name}T{mt}",
                               name=f"{name}T{mt}")
                    for mt in range(D_LOC // P)
                ]
            vpk_sb = [
                const.tile([P, N_HEADS_LOC, 64], dt_mm, tag=f"vp{nt}",
                           name=f"vp{nt}")
                for nt in range(MB)
            ]
            w_sb = {name: [w_tiles[name][:, ts(i, D_LOC)] for i in range(KT)]
                    for name in ("k", "q", "v")}

            def x_chunk(kt, chunk):
                """x.T[128*kt:128*kt+128, CH*chunk:CH*(chunk+1)] view."""
                return xq_sb[chunk][:, ts(kt, CH)]

            def x_block(kt, nt):
                """x.T[128*kt:.., 128*nt:128*nt+128] (one key block)."""
                c, r = divmod(nt, NCH)
                return xq_sb[c][:, kt * CH + r * P:kt * CH + (r + 1) * P]

            # ---- t=0: exp-table primer + PE HAM warm-up ----
            nc.vector.memset(ones_sb[:], 1.0)
            nc.vector.memset(warm_sb[:], 0.0)
            prim = work.tile([P, 8], f32, tag="prim")
            nc.scalar.activation(prim[:], ones_sb[:, 0:8], Exp)
            wps = psum.tile([P, 1024], f32, tag="s", name="warmps")
            for i in range(20):         # bridge from framework start to the
                nc.tensor.matmul(       # boot DMA so the HAM never throttles
                    wps[:, 0:CH], lhsT=warm_sb[:, 0:P],
                    rhs=warm_sb[:], start=True, stop=True)

            # ---- DMA emission ----
            # Everything on the critical path goes on ONE queue in priority
            # order (two active queues round-robin at the SDMA level, which
            # starves whichever one holds the critical transfers), and every
            # big transfer is ~1MB with 8KB contiguous per partition.
            nc.gpsimd.dma_start(out=boot_sb[:], in_=boot[:])
            nc.gpsimd.dma_start(out=wvx1_sb[:], in_=wvx1[:])
            nc.gpsimd.dma_start(out=xq_sb[2][:], in_=xq2[:])
            nc.gpsimd.dma_start(out=xq_sb[3][:], in_=xq3[:])
            for i in range(D_LOC // P):
                nc.gpsimd.dma_start(out=wp_sb[i][:], in_=wpT[ts(i, P), :])
            # biases (tiny, consumed via DVE epilogues) on the other queue
            for name, src_ap in (("q", bq), ("k", bk)):
                for mt in range(D_LOC // P):
                    nc.sync.dma_start(out=bias_sb[name][mt][:],
                                      in_=src_ap[ts(mt, P), :])

            # ---- emission units ----
            def gen_qk(name, mt, h2):
                """q/k projection, one 512-col chunk per "fp" ring slot so
                the DVE epilogue of one chunk overlaps the next chunk's
                matmuls instead of serializing on a shared psum tile."""
                for half in range(2):
                    ps = psum.tile([P, CH], f32, tag="fp", bufs=2,
                                   name=f"fp_{name}{mt}{h2}{half}")
                    for kt in range(KT):
                        nc.tensor.matmul(
                            ps[:],
                            lhsT=w_sb[name][kt][:, ts(mt, P)],
                            rhs=x_chunk(kt, 2 * h2 + half),
                            start=(kt == 0), stop=(kt == KT - 1),
                        )
                        yield
                    nc.vector.tensor_scalar_add(
                        qk_sb[name][mt][:, ts(2 * h2 + half, CH)],
                        ps[:], bias_sb[name][mt][:],
                    )
                    yield

            def gen_v(nts):
                """v-projection groups; one key block per yield.  Shares the
                scores "s" psum ring (runs only while pair 0 is in flight)."""
                for nt in nts:
                    ps = psum.tile([P, 1024], f32, tag="s", name=f"ps_v{nt}")
                    for kt in range(KT):
                        nc.tensor.matmul(
                            ps[:, 0:D_LOC],
                            lhsT=x_block(kt, nt),
                            rhs=w_sb["v"][kt][:],
                            start=(kt == 0), stop=(kt == KT - 1),
                        )
                    nc.vector.tensor_copy(
                        vpk_sb[nt][:].rearrange("p h n -> p (h n)"),
                        ps[:, 0:D_LOC])
                    yield

            def gen_outproj(ch, tag="fp"):
                """Output projection for chunk ch; one 128-row mo slice per
                "fp"/"s" ring slot, evacuated per slice."""
                at_tiles = at_sb[ch]
                for mo in range(DIM // P):
                    pp = psum.tile([P, CH], f32, tag=tag, bufs=2,
                                   name=f"pp{ch}{mo}")
                    for dt_i in range(2):
                        nc.tensor.matmul(
                            pp[:],
                            lhsT=wp_sb[dt_i][:, ts(mo, P)],
                            rhs=at_tiles[dt_i][:],
                            start=(dt_i == 0), stop=(dt_i == 1),
                        )
                    yield
                    os_sb = work.tile([P, CH], dt_mm, tag="os", bufs=4,
                                      name=f"os{ch}{mo}")
                    nc.vector.tensor_copy(os_sb[:], pp[:])
                    nc.sync.dma_start(out=outT[ts(mo, P), ts(ch, CH)],
                                      in_=os_sb[:])
                    yield

            o3p = {}

            def gen_op3_partial():
                """First-head-pair half of outproj chunk 3, staged to SBUF
                during pair 7 so only the second half trails the last exp."""
                at0 = at_sb[3][0]
                for mo in range(DIM // P):
                    pp = psum.tile([P, CH], f32, tag="fp", bufs=2,
                                   name=f"p3a{mo}")
                    nc.tensor.matmul(pp[:], lhsT=wp_sb[0][:, ts(mo, P)],
                                     rhs=at0[:], start=True, stop=True)
                    yield
                    o3p[mo] = work.tile([P, CH], f32, tag="o3p", bufs=8,
                                        name=f"o3p{mo}")
                    nc.vector.tensor_copy(o3p[mo][:], pp[:])
                    yield

            def run_op3_tail():
                at1 = at_sb[3][1]
                for mo in range(DIM // P):
                    pp = psum.tile([P, CH], f32, tag="s", bufs=2,
                                   name=f"p3b{mo}")
                    nc.tensor.matmul(pp[:], lhsT=wp_sb[1][:, ts(mo, P)],
                                     rhs=at1[:], start=True, stop=True)
                    os_sb = work.tile([P, CH], dt_mm, tag="os", bufs=4,
                                      name=f"os3{mo}")
                    nc.vector.tensor_add(os_sb[:], pp[:], o3p[mo][:])
                    nc.sync.dma_start(out=outT[ts(mo, P), ts(3, CH)],
                                      in_=os_sb[:])

            def run(gen):
                for _ in gen:
                    pass

            # ---- preamble: k/q chunk-0 projections, interleaved so the two
            # "fp" ring slots pipeline (k h0 -> slot A, q h0 -> slot B) and
            # the first scores can go right after the two epilogues.
            kh = gen_qk("k", 0, 0)
            qh = gen_qk("q", 0, 0)
            for _ in range(8):
                next(kh)                # k chunk-0 matmuls
            for _ in range(8):
                next(qh)                # q chunk-0 matmuls
            next(kh)                    # k chunk-0 bias epilogue
            next(qh)                    # q chunk-0 bias epilogue
            v_gen = gen_v(list(range(MB)))

            # ---- filler schedule: (start_group, end_group, generator) ----
            # successive "fp" tenancies must be released in emission order.
            at_sb = {}
            fillers = [
                [0, 0, kh, 9],          # k chunk 1 (needed by scores of G1)
                [0, 7, v_gen, 16],
                [0, 3, qh, 9],          # q chunk 1 (needed by pair 1)
                [2, 5, gen_qk("k", 0, 1), 18],
                [5, 8, gen_qk("k", 1, 0), 18],
                [9, 12, gen_qk("k", 1, 1), 18],
                [13, 16, gen_qk("q", 1, 0), 18],
                [17, 20, gen_qk("q", 0, 1), 18],
                [21, 24, gen_qk("q", 1, 1), 18],
                [26, 31, None, 16],     # outproj 0 (created lazily)
                [33, 39, None, 16],     # outproj 1
                [49, 55, None, 16],     # outproj 2
                [57, 62, None, 16],     # outproj 3, first head-pair half
            ]
            op_for_window = {9: 0, 10: 1, 11: 2}

            def pump(G):
                for idx, f in enumerate(fillers):
                    s, e, gen, rem = f
                    if G < s or rem <= 0:
                        continue
                    if gen is None:
                        gen = f[2] = (gen_op3_partial() if idx == 12 else
                                      gen_outproj(op_for_window[idx]))
                    n = rem if G >= e else math.ceil(rem / (e - G + 1))
                    for _ in range(n):
                        if next(gen, StopIteration) is StopIteration:
                            f[3] = 0
                            break
                        f[3] -= 1

            # ---- main loop: 64 groups of 2 key blocks ----
            SEQ = [(0, 0), (1, 0), (0, 1), (1, 1),
                   (2, 0), (2, 1), (3, 0), (3, 1)]
            pts = {}
            pend = {}
            po_pd = {}
            for G in range(NCH * 2 * NG + 1):
                if G < NCH * 2 * NG:
                    p, g = divmod(G, NG)
                    c, hp = SEQ[p]
                    if g == 0:
                        po_pd[p] = (
                            psum_o.tile([P, CH], f32, tag="po", name=f"po{p}"),
                            psum_o.tile([P, CH], f32, tag="po", name=f"pd{p}"),
                        )
                    # leading: scores + exp for blocks 2g, 2g+1 (row2 packed)
                    for j in range(2):
                        mb = 2 * g + j
                        ps = psum.tile([P, 1024], f32, tag="s",
                                       name=f"s{p}_{g}{j}")
                        nc.tensor.matmul(
                            ps[:, 0:CH],
                            lhsT=qk_sb["k"][hp][0:64, ts(mb, P)],
                            rhs=qk_sb["q"][hp][0:64, ts(c, CH)],
                        )
                        nc.tensor.matmul(
                            ps[:, CH:1024],
                            lhsT=qk_sb["k"][hp][64:P, ts(mb, P)],
                            rhs=qk_sb["q"][hp][64:P, ts(c, CH)],
                        )
                        pt = work.tile([P, 1024], dt_mm, tag="pt", bufs=10,
                                       name=f"pt{p}_{g}{j}")
                        nc.scalar.activation(pt[:], ps[:], Exp, scale=SCALE)
                        pts[(G, j)] = pt
                    # denominator pair-sum on the DVE (consumed next group)
                    s2 = work.tile([P, 1024], dt_mm, tag="pts2", bufs=4,
                                   name=f"pts2_{p}_{g}")
                    nc.vector.tensor_add(s2[:], pts[(G, 0)][:], pts[(G, 1)][:])
                    pend[G] = s2
                if G >= 1:
                    # trailing: PV + pd for group G-1 (col2 packed)
                    G2 = G - 1
                    p2, g2 = divmod(G2, NG)
                    c2, hp2 = SEQ[p2]
                    po, pd = po_pd[p2]
                    for j in range(2):
                        mb = 2 * g2 + j
                        pt = pts.pop((G2, j))
                        st = (mb == 0)
                        sp = (mb == MB - 1)
                        nc.tensor.matmul(
                            po[0:64, :], lhsT=vpk_sb[mb][:, 2 * hp2, :],
                            rhs=pt[:, 0:CH], start=st, stop=sp,
                        )
                        nc.tensor.matmul(
                            po[64:P, :], lhsT=vpk_sb[mb][:, 2 * hp2 + 1, :],
                            rhs=pt[:, CH:1024], start=st, stop=sp,
                        )
                    if g2 >= 1:
                        s2p = pend.pop(G2 - 1)
                        nc.tensor.matmul(
                            pd[0:64, :], lhsT=ones_sb[:],
                            rhs=s2p[:, 0:CH], start=(g2 == 1), stop=False,
                        )
                        nc.tensor.matmul(
                            pd[64:P, :], lhsT=ones_sb[:],
                            rhs=s2p[:, CH:1024], start=(g2 == 1), stop=False,
                        )
                    if g2 == NG - 1:
                        # pair p2 done: last pd group + normalize
                        s2p = pend.pop(G2)
                        nc.tensor.matmul(
                            pd[0:64, :], lhsT=ones_sb[:],
                            rhs=s2p[:, 0:CH], start=False, stop=True,
                        )
                        nc.tensor.matmul(
                            pd[64:P, :], lhsT=ones_sb[:],
                            rhs=s2p[:, CH:1024], start=False, stop=True,
                        )
                        del po_pd[p2]
                        rec = work.tile([P, CH], f32, tag="bc", bufs=4,
                                        name=f"rec{p2}")
                        nc.vector.reciprocal_approx_fast(rec[:], pd[:])
                        at = work.tile([P, CH], dt_mm, tag="at", bufs=4,
                                       name=f"at{p2}")
                        nc.vector.tensor_mul(at[:], po[:], rec[:])
                        at_sb.setdefault(c2, []).append(at)
                if G < NCH * 2 * NG:
                    # fillers last: scores stay at the front of the PE queue
                    # so the exp stream never starves (filler windows end one
                    # group before their outputs' first consumers)
                    pump(G)
            # tail: only the second head-pair half of outproj chunk 3 remains
            run_op3_tail()

    nc.compile()
    return nc


def _get_nc():
    if "nc" not in _NC_CACHE:
        _NC_CACHE["nc"] = build_nc(DT_MM_NAME)
    return _NC_CACHE["nc"]


def make_in_maps(x, Wq, bq, Wk, bk, Wv, bv, Wp, bp, dt_mm_name="bfloat16"):
    """Shard full inputs into 8 per-core input maps."""
    f = np.float32
    if dt_mm_name == "bfloat16":
        import ml_dtypes
        mmt = ml_dtypes.bfloat16
    else:
        mmt = np.float32
    x = np.asarray(x, f)
    # chunk-major x: xq[c][p, kt*CH + t] = x[b].T[128*kt + p, CH*c + t]
    xqs = []
    for b in range(x.shape[0]):
        xb = np.ascontiguousarray(x[b].T).astype(mmt)          # [1024, 2048]
        xqs.append(np.ascontiguousarray(
            xb.reshape(KT, P, NCH, CH).transpose(2, 1, 0, 3)
              .reshape(NCH * P, KT * CH)))
    WqT = np.asarray(Wq, f).T
    WkT = np.asarray(Wk, f).T
    WvT = np.asarray(Wv, f).T
    WpT = np.asarray(Wp, f).T
    CAT = np.concatenate
    def pretile(w):
        # [1024, 256] -> [128, 8*256]: partition p holds all 8 k-tiles
        # contiguously so DMA descriptors are 4KB DRAM runs
        return np.ascontiguousarray(
            w.reshape(KT, P, D_LOC).transpose(1, 0, 2).reshape(P, KT * D_LOC)
        ).astype(mmt)

    in_maps = []
    for c in range(N_CORES):
        b, r = divmod(c, 4)
        sl = slice(D_LOC * r, D_LOC * (r + 1))
        xqb = xqs[b]
        in_maps.append({
            "boot": np.ascontiguousarray(
                CAT([pretile(WkT[:, sl]), pretile(WqT[:, sl]),
                     xqb[0:P]], axis=1)),
            "wvx1": np.ascontiguousarray(
                CAT([pretile(WvT[:, sl]), xqb[P:2 * P]], axis=1)),
            "xq2": np.ascontiguousarray(xqb[2 * P:3 * P]),
            "xq3": np.ascontiguousarray(xqb[3 * P:4 * P]),
            "wpT": np.ascontiguousarray(WpT[sl, :]).astype(mmt),
            "bq": np.asarray(bq, f)[sl].reshape(D_LOC, 1).copy(),
            "bk": np.asarray(bk, f)[sl].reshape(D_LOC, 1).copy(),
        })
    return in_maps


def assemble_output(results, Wv, bv, Wp, bp):
    """Sum TP partials, transpose back, add folded biases."""
    f = np.float32
    bp_eff = np.asarray(bv, f) @ np.asarray(Wp, f).T + np.asarray(bp, f)
    out = np.empty((2, N_TOK, DIM), f)
    for b in range(2):
        acc = results[4 * b]["outT"].astype(f)
        for r in range(1, 4):
            acc = acc + results[4 * b + r]["outT"]
        out[b] = acc.T + bp_eff
    return out


DT_MM_NAME = "bfloat16"


def kernel(x, Wq, bq, Wk, bk, Wv, bv, Wp, bp):
    from concourse.bass_utils import run_bass_kernel_spmd
    nc = _get_nc()
    in_maps = make_in_maps(x, Wq, bq, Wk, bk, Wv, bv, Wp, bp, DT_MM_NAME)
    res = run_bass_kernel_spmd(nc, in_maps, list(range(N_CORES)))
    return assemble_output(res.results, Wv, bv, Wp, bp)
